# revision 17
# baseline (speedup 1.0000x reference)
"""GCN (2x GCNConv + linear head) on 8 NeuronCores via Bass/Tile.

v4 strategy (graph/data parallel, PE-centric, skew-tolerant pipeline):
  - Nodes padded to 10240 = 80 blocks of 128; core c owns dst range
    [c*1280, (c+1)*1280).
  - S = D^-1/2 (A+I) D^-1/2 factored so the sparse matrix holds exact
    small-integer counts stored as dense 128-row fp8 blocks (exact 0/1/2),
    contracted on the TensorEngine. Src-side D^-1/2 folded into x on the
    host; dst-side D^-1/2 shipped pre-broadcast as [128, DST] f32.
  - Layer algebra: acc = (A+I)^T (Dinv x);  z = W^T acc;
    h = Dinv relu(z_scaled + b) with per-column Dinv scalings on DVE.
  - L1 runs dst-chunk-outer with FIVE 256-wide chunks: each chunk's g1
    is PE-transposed to node-major and AllGathered immediately, so
    production rate (~10us/chunk) matches L2 consumption rate and the
    collective chain pipelines even under multi-10us core launch skew.
    Gathered slabs are directly usable as lhsT blocks in L2 (no
    post-gather transposes). Chunk hinge PE ops are emitted a few
    blocks into the next chunk's matmul stream to avoid PE bubbles.
  - L2 consumes gathered src blocks in chunk-arrival order, 5 dst-range
    matmuls per block accumulating into 3 PSUM banks; the final
    gathered chunk runs dst-bank-major so W2 + head for finished banks
    overlap the remaining aggregation matmuls.
  - Head (h2 @ Wh + bh) in f32; per-bank output DMA; host trims pads.
"""
import numpy as np
import ml_dtypes

import concourse.bass as bass
import concourse.mybir as mybir
import concourse.tile as tile
import concourse.bacc as bacc
from concourse.bass_utils import run_bass_kernel_spmd

FP8 = np.dtype(ml_dtypes.float8_e4m3)
BF16 = np.dtype(ml_dtypes.bfloat16)

N, E, D, C = 10000, 640000, 128, 40
NCORES = 8
NSB = 80                      # src blocks of 128
NPAD = NSB * 128              # 10240
DST = NPAD // NCORES          # 1280 dst nodes per core
NCH = 5
CW = DST // NCH               # 256-wide AG chunks
CH = [(k * CW, CW) for k in range(NCH)]
BANKS = [(0, 512), (512, 512), (1024, 256)]   # L2 PSUM dst banks
NBC = DST // 128              # node blocks per core (10)

_cache = {}


def _build():
    nc = bacc.Bacc("TRN2", target_bir_lowering=False, debug=False,
                   num_devices=NCORES)
    f32 = mybir.dt.float32
    bf16 = mybir.dt.bfloat16
    fp8 = mybir.dt.float8e4
    RELU = mybir.ActivationFunctionType.Relu

    x_nm = nc.dram_tensor("x_nm", [128, NPAD], bf16, kind="ExternalInput")
    W1b = nc.dram_tensor("W1b", [D, D], bf16, kind="ExternalInput")
    W2b = nc.dram_tensor("W2b", [D, D], bf16, kind="ExternalInput")
    Wh = nc.dram_tensor("Wh", [D, C], f32, kind="ExternalInput")
    b1 = nc.dram_tensor("b1", [D, 1], f32, kind="ExternalInput")
    b2 = nc.dram_tensor("b2", [D, 1], f32, kind="ExternalInput")
    eye = nc.dram_tensor("eye", [128, 128], bf16, kind="ExternalInput")
    bhb_d = nc.dram_tensor("bhb", [128, C], f32, kind="ExternalInput")
    dinvb_d = nc.dram_tensor("dinvb", [128, DST], f32, kind="ExternalInput")
    A_d = [nc.dram_tensor(f"A{ci}", [128, NSB * CW], fp8, kind="ExternalInput")
           for ci in range(NCH)]
    out = nc.dram_tensor("out", [DST, C], f32, kind="ExternalOutput")

    with tile.TileContext(nc) as tc:
        with (
            tc.tile_pool(name="big", bufs=1) as big,
            tc.tile_pool(name="sb", bufs=1) as sb,
            tc.tile_pool(name="tmpp", bufs=3) as tmpp,
            tc.tile_pool(name="psl1", bufs=2, space="PSUM") as psl1,
            tc.tile_pool(name="psl2", bufs=1, space="PSUM") as psl2,
            tc.tile_pool(name="psz", bufs=1, space="PSUM") as psz,
            tc.tile_pool(name="pstr", bufs=1, space="PSUM") as pstr,
            tc.tile_pool(name="psmisc", bufs=1, space="PSUM") as psmisc,
            tc.tile_pool(name="dram", bufs=1, space="DRAM") as dram,
        ):
            # ---- warm-up collective: absorbs the cross-core entry skew
            # and comm bootstrap under layer-1 compute ----
            warm_in = dram.tile([1, 128], bf16)
            warm_out = dram.tile([NCORES, 1, 128], bf16, addr_space="Shared")
            nc.gpsimd.collective_compute(
                "AllGather", mybir.AluOpType.bypass,
                replica_groups=[list(range(NCORES))],
                ins=[warm_in[:]], outs=[warm_out[:]])

            # ---- small inputs first on the sync queue ----
            W1_t = sb.tile([D, D], bf16)
            nc.sync.dma_start(W1_t[:], W1b[:, :])
            W2_t = sb.tile([D, D], bf16)
            nc.sync.dma_start(W2_t[:], W2b[:, :])
            Wh_t = sb.tile([D, C], f32)
            nc.sync.dma_start(Wh_t[:], Wh[:, :])
            b1_t = sb.tile([D, 1], f32)
            nc.sync.dma_start(b1_t[:], b1[:, :])
            b2_t = sb.tile([D, 1], f32)
            nc.sync.dma_start(b2_t[:], b2[:, :])
            eye_t = sb.tile([128, 128], bf16)
            nc.sync.dma_start(eye_t[:], eye[:, :])
            bhb = sb.tile([128, C], f32)
            nc.sync.dma_start(bhb[:], bhb_d[:, :])

            # ---- big streams: A chunk 0 first, then dinvb, then the rest.
            # x streams straight into g0 on the scalar queue concurrently.
            A_t = [big.tile([128, NSB * CW], fp8, name=f"At{ci}")
                   for ci in range(NCH)]
            for q in range(8):
                s0 = q * 10 * CW
                s1 = (q + 1) * 10 * CW
                nc.sync.dma_start(A_t[0][:, s0:s1], A_d[0][:, s0:s1])
            dinvb = sb.tile([128, DST], f32)
            nc.sync.dma_start(dinvb[:], dinvb_d[:, :])
            for ci in range(1, NCH):
                for q in range(4):
                    s0 = q * 20 * CW
                    s1 = (q + 1) * 20 * CW
                    nc.sync.dma_start(A_t[ci][:, s0:s1], A_d[ci][:, s0:s1])

            g0 = big.tile([128, NPAD], bf16)
            for p in range(8):
                s0 = p * 10 * 128
                s1 = (p + 1) * 10 * 128
                nc.scalar.dma_start(g0[:, s0:s1], x_nm[:, s0:s1])

            # ---- layer 1: chunk-outer aggregation, hinges interleaved ----
            g1nm = []     # gathered node-major slabs, one per chunk

            def hinge(ci):
                """After chunk ci's agg bank is full: W1, scalings, relu,
                transpose to node-major, fire the chunk's AllGather."""
                off = CH[ci][0]
                nt = CW // 128
                agg = agg_banks[ci]
                acc = tmpp.tile([128, CW], bf16, tag="acc")
                nc.vector.tensor_copy(acc[:], agg[:])
                zps = psz.tile([128, 512], f32, tag="z")
                nc.tensor.matmul(zps[:, :CW], lhsT=W1_t[:], rhs=acc[:],
                                 start=True, stop=True)
                t1 = tmpp.tile([128, CW], f32, tag="t1")
                nc.vector.tensor_mul(t1[:], zps[:, :CW],
                                     dinvb[:, off:off + CW])
                t2 = tmpp.tile([128, CW], f32, tag="t2")
                nc.scalar.activation(t2[:], t1[:], RELU,
                                     bias=b1_t[:, 0:1], scale=1.0)
                g1c = tmpp.tile([128, CW], bf16, tag="g1c")
                nc.vector.tensor_mul(g1c[:], t2[:], dinvb[:, off:off + CW])
                # transpose own chunk to node-major BEFORE the collective
                trp = pstr.tile([128, 2, 128], bf16, tag="tr")
                for t in range(nt):
                    nc.tensor.transpose(trp[:, t, :],
                                        g1c[:, t * 128:(t + 1) * 128],
                                        eye_t[:])
                cin = tmpp.tile([128, CW], bf16, tag="cin")
                nc.vector.tensor_copy(
                    cin[:].rearrange("p (t f) -> p t f", f=128),
                    trp[:, :nt, :])
                cc_in = dram.tile([128, CW], bf16, name=f"cc_in{ci}")
                cc_out = dram.tile([NCORES, 128, CW], bf16,
                                   addr_space="Shared", name=f"cc_out{ci}")
                nc.scalar.dma_start(cc_in[:], cin[:])
                nc.gpsimd.collective_compute(
                    "AllGather", mybir.AluOpType.bypass,
                    replica_groups=[list(range(NCORES))],
                    ins=[cc_in[:]], outs=[cc_out[:]])
                # gathered slab straight to SBUF, already node-major
                gnm = big.tile([128, NCORES * CW], bf16, name=f"g1nm{ci}")
                nc.sync.dma_start(
                    gnm[:].rearrange("p (r d) -> p r d", d=CW),
                    cc_out[:, :, :].rearrange("r p d -> p r d"))
                g1nm.append(gnm)

            agg_banks = []
            for ci in range(NCH):
                agg = psl1.tile([128, CW], f32, tag="agg", name=f"agg1_{ci}")
                agg_banks.append(agg)
                for sbk in range(NSB):
                    if ci > 0 and sbk == 8:
                        hinge(ci - 1)
                    nc.tensor.matmul(
                        agg[:], lhsT=g0[:, sbk * 128:(sbk + 1) * 128],
                        rhs=A_t[ci][:, sbk * CW:(sbk + 1) * CW],
                        start=(sbk == 0), stop=(sbk == NSB - 1))

            # ---- layer 2: src blocks in chunk-arrival order ----
            # 3 PSUM dst banks; the 5 256-wide A chunks map into them.
            agg2 = [psl2.tile([128, ln], f32, tag=f"agg2_{b}",
                              name=f"agg2_{b}")
                    for b, (off, ln) in enumerate(BANKS)]

            def l2_block(lhsT, sb_g, k):
                for c in range(NCH):
                    boff = c * CW
                    b = boff // 512
                    lo = boff - BANKS[b][0]
                    # start=True clears has_written for the WHOLE bank, so
                    # only the first sub-range of a shared bank may set it;
                    # the second sub-range's first write lands on cleared
                    # bits and overwrites (exactly what we need).
                    nc.tensor.matmul(
                        agg2[b][:, lo:lo + CW], lhsT=lhsT,
                        rhs=A_t[c][:, sb_g * CW:(sb_g + 1) * CW],
                        start=(k == 0 and lo == 0), stop=False)

            def blocks_of(ci):
                gnm = g1nm[ci]
                for r in range(NCORES):
                    for t in range(CW // 128):
                        lhsT = gnm[:, (r * 2 + t) * 128:(r * 2 + t + 1) * 128]
                        yield lhsT, r * NBC + ci * 2 + t

            h2 = sb.tile([128, DST], f32)

            def tail(b):
                """agg2 bank b complete: W2, scalings, relu, head, out."""
                off2, ln2 = BANKS[b]
                nt2 = ln2 // 128
                acc = tmpp.tile([128, 512], bf16, tag="acc2")
                nc.vector.tensor_copy(acc[:, :ln2], agg2[b][:, :ln2])
                zps = psz.tile([128, 512], f32, tag="z")
                nc.tensor.matmul(zps[:, :ln2], lhsT=W2_t[:], rhs=acc[:, :ln2],
                                 start=True, stop=True)
                t1 = tmpp.tile([128, 512], f32, tag="t12")
                nc.vector.tensor_mul(t1[:, :ln2], zps[:, :ln2],
                                     dinvb[:, off2:off2 + ln2])
                nc.scalar.activation(h2[:, off2:off2 + ln2], t1[:, :ln2],
                                     RELU, bias=b2_t[:, 0:1], scale=1.0)
                outc = tmpp.tile([128, 4 * C], f32, tag="outc")
                for t in range(nt2):
                    hk = off2 // 128 + t
                    hd = psmisc.tile([128, C], f32, tag="hd")
                    nc.tensor.matmul(hd[:],
                                     lhsT=h2[:, hk * 128:(hk + 1) * 128],
                                     rhs=Wh_t[:], start=True, stop=True)
                    nc.vector.tensor_add(outc[:, t * C:(t + 1) * C], hd[:],
                                         bhb[:, :])
                nc.scalar.dma_start(
                    out[off2:off2 + ln2, :].rearrange("(t p) c -> p t c",
                                                      p=128),
                    outc[:, :nt2 * C].rearrange("p (t c) -> p t c", c=C))

            # gathered chunks 0..3: straight consumption; the L1 last-chunk
            # hinge rides a few blocks into chunk 0's stream.
            k = 0
            for ci in range(NCH - 1):
                for lhsT, sb_g in blocks_of(ci):
                    if ci == 0 and k == 8:
                        hinge(NCH - 1)
                    l2_block(lhsT, sb_g, k)
                    k += 1
            # final gathered chunk: dst-major with tails staggered in.
            fin = list(blocks_of(NCH - 1))
            for c in range(NCH):
                boff = c * CW
                b = boff // 512
                lo = boff - BANKS[b][0]
                for j, (lhsT, sb_g) in enumerate(fin):
                    nc.tensor.matmul(
                        agg2[b][:, lo:lo + CW], lhsT=lhsT,
                        rhs=A_t[c][:, sb_g * CW:(sb_g + 1) * CW],
                        start=False, stop=(j == len(fin) - 1))
                if c == 1:
                    tail(0)
                elif c == 3:
                    tail(1)
            tail(2)
    nc.compile()
    return nc


def _prep(x, edge_index, W1, b1, W2, b2, Wh, bh):
    x = np.asarray(x, np.float32)
    ei = np.asarray(edge_index, np.int64)
    src = np.concatenate([ei[0], np.arange(NPAD, dtype=np.int64)])
    dst = np.concatenate([ei[1], np.arange(NPAD, dtype=np.int64)])
    deg = np.bincount(dst, minlength=NPAD).astype(np.float32)
    dinv = np.where(deg > 0, 1.0 / np.sqrt(np.maximum(deg, 1.0)),
                    0.0).astype(np.float32)

    xp = np.zeros((NPAD, D), np.float32)
    xp[:N] = x
    xp *= dinv[:, None]         # fold src-side scaling into x on the host
    x_nm = xp.reshape(NSB, 128, D).transpose(1, 0, 2).reshape(128, NPAD)

    shared = {
        "x_nm": x_nm.astype(BF16),
        "W1b": np.asarray(W1, np.float32).astype(BF16),
        "W2b": np.asarray(W2, np.float32).astype(BF16),
        "Wh": np.asarray(Wh, np.float32),
        "b1": np.asarray(b1, np.float32).reshape(D, 1),
        "b2": np.asarray(b2, np.float32).reshape(D, 1),
        "bhb": np.ascontiguousarray(np.broadcast_to(
            np.asarray(bh, np.float32).reshape(1, C), (128, C))),
        "eye": np.eye(128, dtype=np.float32).astype(BF16),
    }
    core = dst // DST
    sl, sbk = src % 128, src // 128
    in_maps = []
    for c in range(NCORES):
        m = core == c
        dloc = dst[m] - c * DST
        im = dict(shared, dinvb=np.ascontiguousarray(np.broadcast_to(
            dinv[c * DST:(c + 1) * DST], (128, DST))))
        for ci, (off, ln) in enumerate(CH):
            m2 = (dloc >= off) & (dloc < off + ln)
            Ac = np.zeros((128, NSB * ln), np.float32)
            np.add.at(Ac, (sl[m][m2], sbk[m][m2] * ln + dloc[m2] - off), 1.0)
            im[f"A{ci}"] = Ac.astype(FP8)
        in_maps.append(im)
    return in_maps


def _run(inputs, trace=False):
    if "nc" not in _cache:
        _cache["nc"] = _build()
    in_maps = _prep(**inputs)
    res = run_bass_kernel_spmd(_cache["nc"], in_maps,
                               core_ids=list(range(NCORES)), trace=trace)
    out = np.concatenate([res.results[c]["out"] for c in range(NCORES)],
                         axis=0)[:N]
    return np.ascontiguousarray(out, dtype=np.float32), res


def kernel(**inputs):
    out, _ = _run(inputs, trace=False)
    return out


# revision 18
# speedup vs baseline: 1.1611x; 1.1611x over previous
"""GCN (2x GCNConv + linear head) on 8 NeuronCores via Bass/Tile.

v4 strategy (graph/data parallel, PE-centric, skew-tolerant pipeline):
  - Nodes padded to 10240 = 80 blocks of 128; core c owns dst range
    [c*1280, (c+1)*1280).
  - S = D^-1/2 (A+I) D^-1/2 factored so the sparse matrix holds exact
    small-integer counts stored as dense 128-row fp8 blocks (exact 0/1/2),
    contracted on the TensorEngine. Src-side D^-1/2 folded into x on the
    host; dst-side D^-1/2 shipped pre-broadcast as [128, DST] f32.
  - Layer algebra: acc = (A+I)^T (Dinv x);  z = W^T acc;
    h = Dinv relu(z_scaled + b) with per-column Dinv scalings on DVE.
  - L1 runs dst-chunk-outer with FIVE 256-wide chunks: each chunk's g1
    is PE-transposed to node-major and AllGathered immediately, so
    production rate (~10us/chunk) matches L2 consumption rate and the
    collective chain pipelines even under multi-10us core launch skew.
    Gathered slabs are directly usable as lhsT blocks in L2 (no
    post-gather transposes). Chunk hinge PE ops are emitted a few
    blocks into the next chunk's matmul stream to avoid PE bubbles.
  - L2 consumes gathered src blocks in chunk-arrival order, 5 dst-range
    matmuls per block accumulating into 3 PSUM banks; the final
    gathered chunk runs dst-bank-major so W2 + head for finished banks
    overlap the remaining aggregation matmuls.
  - Head (h2 @ Wh + bh) in f32; per-bank output DMA; host trims pads.
"""
import numpy as np
import ml_dtypes

import concourse.bass as bass
import concourse.mybir as mybir
import concourse.tile as tile
import concourse.bacc as bacc
from concourse.bass_utils import run_bass_kernel_spmd

FP8 = np.dtype(ml_dtypes.float8_e4m3)
BF16 = np.dtype(ml_dtypes.bfloat16)

N, E, D, C = 10000, 640000, 128, 40
NCORES = 8
NSB = 80                      # src blocks of 128
NPAD = NSB * 128              # 10240
DST = NPAD // NCORES          # 1280 dst nodes per core
NCH = 5
CW = DST // NCH               # 256-wide AG chunks
CH = [(k * CW, CW) for k in range(NCH)]
BANKS = [(0, 512), (512, 512), (1024, 256)]   # L2 PSUM dst banks
NBC = DST // 128              # node blocks per core (10)

_cache = {}


def _build():
    nc = bacc.Bacc("TRN2", target_bir_lowering=False, debug=False,
                   num_devices=NCORES)
    f32 = mybir.dt.float32
    bf16 = mybir.dt.bfloat16
    fp8 = mybir.dt.float8e4
    RELU = mybir.ActivationFunctionType.Relu

    x_nm = nc.dram_tensor("x_nm", [128, NPAD], bf16, kind="ExternalInput")
    W1b = nc.dram_tensor("W1b", [D, D], bf16, kind="ExternalInput")
    W2b = nc.dram_tensor("W2b", [D, D], bf16, kind="ExternalInput")
    Wh = nc.dram_tensor("Wh", [D, C], f32, kind="ExternalInput")
    b1 = nc.dram_tensor("b1", [D, 1], f32, kind="ExternalInput")
    b2 = nc.dram_tensor("b2", [D, 1], f32, kind="ExternalInput")
    eye = nc.dram_tensor("eye", [128, 128], bf16, kind="ExternalInput")
    bhb_d = nc.dram_tensor("bhb", [128, C], f32, kind="ExternalInput")
    dinvb_d = nc.dram_tensor("dinvb", [128, DST], f32, kind="ExternalInput")
    A_d = [nc.dram_tensor(f"A{ci}", [128, NSB * CW], fp8, kind="ExternalInput")
           for ci in range(NCH)]
    out = nc.dram_tensor("out", [DST, C], f32, kind="ExternalOutput")

    with tile.TileContext(nc) as tc:
        with (
            tc.tile_pool(name="big", bufs=1) as big,
            tc.tile_pool(name="sb", bufs=1) as sb,
            tc.tile_pool(name="tmpp", bufs=3) as tmpp,
            tc.tile_pool(name="psl1", bufs=2, space="PSUM") as psl1,
            tc.tile_pool(name="psl2", bufs=1, space="PSUM") as psl2,
            tc.tile_pool(name="psz", bufs=1, space="PSUM") as psz,
            tc.tile_pool(name="pstr", bufs=1, space="PSUM") as pstr,
            tc.tile_pool(name="psmisc", bufs=1, space="PSUM") as psmisc,
            tc.tile_pool(name="dram", bufs=1, space="DRAM") as dram,
        ):
            # ---- A stream alone on the sync queue (descriptor issue is
            # ~0.6us per dma_start per sequencer, so spread inputs across
            # queues). x streams into g0 on the scalar queue; smalls and
            # dinvb go on the gpsimd queue ahead of the AG triggers.
            A_t = [big.tile([128, NSB * CW], fp8, name=f"At{ci}")
                   for ci in range(NCH)]
            for q in range(4):
                s0 = q * 20 * CW
                s1 = (q + 1) * 20 * CW
                nc.sync.dma_start(A_t[0][:, s0:s1], A_d[0][:, s0:s1])
            for ci in range(1, NCH):
                for q in range(2):
                    s0 = q * 40 * CW
                    s1 = (q + 1) * 40 * CW
                    nc.sync.dma_start(A_t[ci][:, s0:s1], A_d[ci][:, s0:s1])

            W1_t = sb.tile([D, D], bf16)
            nc.gpsimd.dma_start(W1_t[:], W1b[:, :])
            b1_t = sb.tile([D, 1], f32)
            nc.gpsimd.dma_start(b1_t[:], b1[:, :])
            eye_t = sb.tile([128, 128], bf16)
            nc.gpsimd.dma_start(eye_t[:], eye[:, :])
            dinvb = sb.tile([128, DST], f32)
            nc.gpsimd.dma_start(dinvb[:], dinvb_d[:, :])
            W2_t = sb.tile([D, D], bf16)
            nc.gpsimd.dma_start(W2_t[:], W2b[:, :])
            b2_t = sb.tile([D, 1], f32)
            nc.gpsimd.dma_start(b2_t[:], b2[:, :])
            Wh_t = sb.tile([D, C], f32)
            nc.gpsimd.dma_start(Wh_t[:], Wh[:, :])
            bhb = sb.tile([128, C], f32)
            nc.gpsimd.dma_start(bhb[:], bhb_d[:, :])

            g0 = big.tile([128, NPAD], bf16)
            for p in range(8):
                s0 = p * 10 * 128
                s1 = (p + 1) * 10 * 128
                nc.scalar.dma_start(g0[:, s0:s1], x_nm[:, s0:s1])

            # ---- layer 1: chunk-outer aggregation, hinges interleaved ----
            g1nm = []     # gathered node-major slabs, one per chunk

            def hinge(ci):
                """After chunk ci's agg bank is full: W1, scalings, relu,
                transpose to node-major, fire the chunk's AllGather."""
                off = CH[ci][0]
                nt = CW // 128
                agg = agg_banks[ci]
                acc = tmpp.tile([128, CW], bf16, tag="acc")
                nc.vector.tensor_copy(acc[:], agg[:])
                zps = psz.tile([128, 512], f32, tag="z")
                nc.tensor.matmul(zps[:, :CW], lhsT=W1_t[:], rhs=acc[:],
                                 start=True, stop=True)
                t1 = tmpp.tile([128, CW], f32, tag="t1")
                nc.vector.tensor_mul(t1[:], zps[:, :CW],
                                     dinvb[:, off:off + CW])
                t2 = tmpp.tile([128, CW], f32, tag="t2")
                nc.scalar.activation(t2[:], t1[:], RELU,
                                     bias=b1_t[:, 0:1], scale=1.0)
                g1c = tmpp.tile([128, CW], bf16, tag="g1c")
                nc.vector.tensor_mul(g1c[:], t2[:], dinvb[:, off:off + CW])
                # transpose own chunk to node-major BEFORE the collective
                trp = pstr.tile([128, 2, 128], bf16, tag="tr")
                for t in range(nt):
                    nc.tensor.transpose(trp[:, t, :],
                                        g1c[:, t * 128:(t + 1) * 128],
                                        eye_t[:])
                cin = tmpp.tile([128, CW], bf16, tag="cin")
                nc.vector.tensor_copy(
                    cin[:].rearrange("p (t f) -> p t f", f=128),
                    trp[:, :nt, :])
                cc_in = dram.tile([128, CW], bf16, name=f"cc_in{ci}")
                cc_out = dram.tile([NCORES, 128, CW], bf16,
                                   addr_space="Shared", name=f"cc_out{ci}")
                nc.scalar.dma_start(cc_in[:], cin[:])
                nc.gpsimd.collective_compute(
                    "AllGather", mybir.AluOpType.bypass,
                    replica_groups=[list(range(NCORES))],
                    ins=[cc_in[:]], outs=[cc_out[:]])
                # gathered slab straight to SBUF, already node-major
                gnm = big.tile([128, NCORES * CW], bf16, name=f"g1nm{ci}")
                nc.sync.dma_start(
                    gnm[:].rearrange("p (r d) -> p r d", d=CW),
                    cc_out[:, :, :].rearrange("r p d -> p r d"))
                g1nm.append(gnm)

            agg_banks = []
            for ci in range(NCH):
                agg = psl1.tile([128, CW], f32, tag="agg", name=f"agg1_{ci}")
                agg_banks.append(agg)
                for sbk in range(NSB):
                    if ci > 0 and sbk == 8:
                        hinge(ci - 1)
                    nc.tensor.matmul(
                        agg[:], lhsT=g0[:, sbk * 128:(sbk + 1) * 128],
                        rhs=A_t[ci][:, sbk * CW:(sbk + 1) * CW],
                        start=(sbk == 0), stop=(sbk == NSB - 1))

            # ---- layer 2: src blocks in chunk-arrival order ----
            # 3 PSUM dst banks; the 5 256-wide A chunks map into them.
            agg2 = [psl2.tile([128, ln], f32, tag=f"agg2_{b}",
                              name=f"agg2_{b}")
                    for b, (off, ln) in enumerate(BANKS)]

            def l2_block(lhsT, sb_g, k):
                for c in range(NCH):
                    boff = c * CW
                    b = boff // 512
                    lo = boff - BANKS[b][0]
                    # start=True clears has_written for the WHOLE bank, so
                    # only the first sub-range of a shared bank may set it;
                    # the second sub-range's first write lands on cleared
                    # bits and overwrites (exactly what we need).
                    nc.tensor.matmul(
                        agg2[b][:, lo:lo + CW], lhsT=lhsT,
                        rhs=A_t[c][:, sb_g * CW:(sb_g + 1) * CW],
                        start=(k == 0 and lo == 0), stop=False)

            def blocks_of(ci):
                gnm = g1nm[ci]
                for r in range(NCORES):
                    for t in range(CW // 128):
                        lhsT = gnm[:, (r * 2 + t) * 128:(r * 2 + t + 1) * 128]
                        yield lhsT, r * NBC + ci * 2 + t

            h2 = sb.tile([128, DST], f32)

            def tail(b):
                """agg2 bank b complete: W2, scalings, relu, head, out."""
                off2, ln2 = BANKS[b]
                nt2 = ln2 // 128
                acc = tmpp.tile([128, 512], bf16, tag="acc2")
                nc.vector.tensor_copy(acc[:, :ln2], agg2[b][:, :ln2])
                zps = psz.tile([128, 512], f32, tag="z")
                nc.tensor.matmul(zps[:, :ln2], lhsT=W2_t[:], rhs=acc[:, :ln2],
                                 start=True, stop=True)
                t1 = tmpp.tile([128, 512], f32, tag="t12")
                nc.vector.tensor_mul(t1[:, :ln2], zps[:, :ln2],
                                     dinvb[:, off2:off2 + ln2])
                nc.scalar.activation(h2[:, off2:off2 + ln2], t1[:, :ln2],
                                     RELU, bias=b2_t[:, 0:1], scale=1.0)
                outc = tmpp.tile([128, 4 * C], f32, tag="outc")
                for t in range(nt2):
                    hk = off2 // 128 + t
                    hd = psmisc.tile([128, C], f32, tag="hd")
                    nc.tensor.matmul(hd[:],
                                     lhsT=h2[:, hk * 128:(hk + 1) * 128],
                                     rhs=Wh_t[:], start=True, stop=True)
                    nc.vector.tensor_add(outc[:, t * C:(t + 1) * C], hd[:],
                                         bhb[:, :])
                nc.scalar.dma_start(
                    out[off2:off2 + ln2, :].rearrange("(t p) c -> p t c",
                                                      p=128),
                    outc[:, :nt2 * C].rearrange("p (t c) -> p t c", c=C))

            # gathered chunks 0..3: straight consumption; the L1 last-chunk
            # hinge rides a few blocks into chunk 0's stream.
            k = 0
            for ci in range(NCH - 1):
                for lhsT, sb_g in blocks_of(ci):
                    if ci == 0 and k == 8:
                        hinge(NCH - 1)
                    l2_block(lhsT, sb_g, k)
                    k += 1
            # final gathered chunk: dst-major with tails staggered in.
            fin = list(blocks_of(NCH - 1))
            for c in range(NCH):
                boff = c * CW
                b = boff // 512
                lo = boff - BANKS[b][0]
                for j, (lhsT, sb_g) in enumerate(fin):
                    nc.tensor.matmul(
                        agg2[b][:, lo:lo + CW], lhsT=lhsT,
                        rhs=A_t[c][:, sb_g * CW:(sb_g + 1) * CW],
                        start=False, stop=(j == len(fin) - 1))
                if c == 1:
                    tail(0)
                elif c == 3:
                    tail(1)
            tail(2)
    nc.compile()
    return nc


def _prep(x, edge_index, W1, b1, W2, b2, Wh, bh):
    x = np.asarray(x, np.float32)
    ei = np.asarray(edge_index, np.int64)
    src = np.concatenate([ei[0], np.arange(NPAD, dtype=np.int64)])
    dst = np.concatenate([ei[1], np.arange(NPAD, dtype=np.int64)])
    deg = np.bincount(dst, minlength=NPAD).astype(np.float32)
    dinv = np.where(deg > 0, 1.0 / np.sqrt(np.maximum(deg, 1.0)),
                    0.0).astype(np.float32)

    xp = np.zeros((NPAD, D), np.float32)
    xp[:N] = x
    xp *= dinv[:, None]         # fold src-side scaling into x on the host
    x_nm = xp.reshape(NSB, 128, D).transpose(1, 0, 2).reshape(128, NPAD)

    shared = {
        "x_nm": x_nm.astype(BF16),
        "W1b": np.asarray(W1, np.float32).astype(BF16),
        "W2b": np.asarray(W2, np.float32).astype(BF16),
        "Wh": np.asarray(Wh, np.float32),
        "b1": np.asarray(b1, np.float32).reshape(D, 1),
        "b2": np.asarray(b2, np.float32).reshape(D, 1),
        "bhb": np.ascontiguousarray(np.broadcast_to(
            np.asarray(bh, np.float32).reshape(1, C), (128, C))),
        "eye": np.eye(128, dtype=np.float32).astype(BF16),
    }
    core = dst // DST
    sl, sbk = src % 128, src // 128
    in_maps = []
    for c in range(NCORES):
        m = core == c
        dloc = dst[m] - c * DST
        im = dict(shared, dinvb=np.ascontiguousarray(np.broadcast_to(
            dinv[c * DST:(c + 1) * DST], (128, DST))))
        for ci, (off, ln) in enumerate(CH):
            m2 = (dloc >= off) & (dloc < off + ln)
            Ac = np.zeros((128, NSB * ln), np.float32)
            np.add.at(Ac, (sl[m][m2], sbk[m][m2] * ln + dloc[m2] - off), 1.0)
            im[f"A{ci}"] = Ac.astype(FP8)
        in_maps.append(im)
    return in_maps


def _run(inputs, trace=False):
    if "nc" not in _cache:
        _cache["nc"] = _build()
    in_maps = _prep(**inputs)
    res = run_bass_kernel_spmd(_cache["nc"], in_maps,
                               core_ids=list(range(NCORES)), trace=trace)
    out = np.concatenate([res.results[c]["out"] for c in range(NCORES)],
                         axis=0)[:N]
    return np.ascontiguousarray(out, dtype=np.float32), res


def kernel(**inputs):
    out, _ = _run(inputs, trace=False)
    return out
